# revision 3
# baseline (speedup 1.0000x reference)
"""Charge-equilibration (QEq) Bass kernel for Trainium2, 8-core data-parallel.

Algorithm (per molecule, batched 128-per-partition-block on each core):
  - Build the screened-Coulomb matrix C (64x64, SPD; identity rows for
    padded atoms) plus two RHS columns appended to each matrix row:
    u-rhs = valid mask, v-rhs = -electronegativity (masked).
  - Factor C = L D L^T in-place (right-looking, rank-1 updates via a single
    scalar_tensor_tensor outer-product instruction per step).  The RHS
    columns ride along in the trailing updates, so forward substitution is
    free.  The strict upper triangle ends up holding L^T (the "w" rows).
  - Diagonal scale + backward substitution give u = C^-1 m, v = C^-1 (-chi).
  - Lagrange multiplier via the Schur complement of the KKT system:
    lam = (sum(v) - Q_tot) / sum(u);  q = v - lam * u.
  - Energy without materializing the pair matrix, via the KKT identity:
    E = -0.5 * (sum(q*chi) + lam*Q_tot + BOHR * sum(hardness * q^2)).
"""

import numpy as np

BOHR = 0.5291772109
SQRT_PI = 1.7724538509055159

B_TOT = 2048
M = 64
NCORES = 8
NB = B_TOT // NCORES          # molecules per core
P = 128                       # molecules per block (partition dim)
NBLK = NB // P                # blocks per core
W = M + 2                     # augmented row width (64 matrix + 2 rhs)

_prog = None


def _split_multiwait(nc, maxw=1):
    """This toolchain's walrus codegen rejects instructions carrying more
    than one sem-wait; split excess waits onto NOPs inserted just before."""
    from concourse import mybir

    cnt = 0
    for f in nc.m.functions:
        for b in f.blocks:
            il = b.instructions
            i = 0
            while i < len(il):
                ins = il[i]
                si = ins.sync_info
                if si is None:
                    i += 1
                    continue
                w = list(si.on_wait)
                if len(w) <= maxw:
                    i += 1
                    continue
                keep = w[-maxw:]
                extra = w[:-maxw]
                si.on_wait = keep
                at = i
                for j in range(0, len(extra), maxw):
                    nop = mybir.InstNoOp(name=f"Wsplit{cnt}", ins=[], outs=[])
                    cnt += 1
                    nop.engine = ins.engine
                    nop.sync_info = mybir.SyncInfo(
                        on_wait=extra[j : j + maxw], on_update=[]
                    )
                    il.insert(at, nop)
                    at += 1
                i = at + 1
    return cnt


def _sap(ap, offset, dims):
    """Custom strided AP over a tile: keep partition dim, set free dims to
    the given (step, count) pairs and the element offset."""
    c = ap.copy()
    l = c.ap
    while len(l) > 1:
        del l[1]
    for d in dims:
        l.append(list(d))
    c.offset = offset
    return c


def _emit_block(nc, pool, blk, dram):
    import concourse.bass as bass  # noqa: F401
    from concourse import mybir

    OP = mybir.AluOpType
    AF = mybir.ActivationFunctionType
    F = mybir.dt.float32
    V = nc.vector
    S = nc.scalar
    d_pos, d_msk, d_hrd, d_chi, d_sig, d_qt, d_charge, d_energy = dram
    r0 = blk * P
    t = lambda shape, name: pool.tile(shape, F, name=f"{name}_b{blk}", tag=f"{name}_b{blk}")

    pos = t([P, 192], "pos")
    msk = t([P, M], "msk")
    hrd = t([P, M], "hrd")
    chi = t([P, M], "chi")
    sig = t([P, M], "sig")
    qt = t([P, 1], "qt")
    A = t([P, M * W], "A")
    big1 = t([P, 4096], "big1")
    big2 = t([P, 4096], "big2")
    big3 = t([P, 4096], "big3")
    invd = t([P, M], "invd")
    x = t([P, 2 * M], "x")
    s2 = t([P, M], "s2")
    rsig = t([P, M], "rsig")
    diagc = t([P, M], "diagc")
    mB = t([P, M], "mB")
    junk = t([P, M], "junk")
    tq = t([P, M], "tq")
    q = t([P, M], "q")
    e1 = t([P, 1], "e1")
    e2 = t([P, 1], "e2")
    sums2 = t([P, 2], "sums2")
    lam = t([P, 1], "lam")
    neglam = t([P, 1], "neglam")
    su_r = t([P, 1], "sur")
    negx = t([P, 2], "negx")
    en = t([P, 1], "en")

    dma = nc.sync.dma_start
    dma(pos[:], d_pos[r0 : r0 + P, :])
    dma(msk[:], d_msk[r0 : r0 + P, :])
    dma(hrd[:], d_hrd[r0 : r0 + P, :])
    dma(chi[:], d_chi[r0 : r0 + P, :])
    dma(sig[:], d_sig[r0 : r0 + P, :])
    dma(qt[:], d_qt[r0 : r0 + P, :])

    # ---- small precompute ----
    V.tensor_tensor(s2[:], sig[:], sig[:], OP.mult)
    V.reciprocal(rsig[:], sig[:])
    # diagc = msk*((hrd + rsig/sqrt(pi))*BOHR - 1) + 1
    V.scalar_tensor_tensor(diagc[:], rsig[:], 1.0 / SQRT_PI, hrd[:], OP.mult, OP.add)
    V.tensor_scalar(diagc[:], diagc[:], BOHR, 1.0, OP.mult, OP.subtract)
    V.tensor_tensor(diagc[:], diagc[:], msk[:], OP.mult)
    V.tensor_scalar(diagc[:], diagc[:], 1.0, None, OP.add)
    V.tensor_scalar(mB[:], msk[:], BOHR, None, OP.mult)

    # ---- pairwise screened-Coulomb matrix ----
    m3 = lambda tt: tt[:, :].rearrange("p (i j) -> p i j", j=M)
    px = pos[:, :].rearrange("p (a c) -> p a c", c=3)
    for c in range(3):
        xi = px[:, :, c]
        V.tensor_tensor(
            m3(big1),
            xi[:, :, None].broadcast_to([P, M, M]),
            xi[:, None, :].broadcast_to([P, M, M]),
            OP.subtract,
        )
        if c == 0:
            V.tensor_tensor(big2[:], big1[:], big1[:], OP.mult)
        elif c == 1:
            V.tensor_tensor(big3[:], big1[:], big1[:], OP.mult)
            V.tensor_tensor(big2[:], big2[:], big3[:], OP.add)
        else:
            V.tensor_tensor(big3[:], big1[:], big1[:], OP.mult)
            # (dz^2 + eps) + sq — eps keeps the (masked) diagonal finite
            V.scalar_tensor_tensor(big2[:], big3[:], 1e-12, big2[:], OP.add, OP.add)
    S.activation(big1[:], big2[:], AF.Ln)               # ln(sq + eps)
    S.activation(big3[:], big1[:], AF.Exp, scale=0.5)   # d
    S.activation(big2[:], big1[:], AF.Exp, scale=-0.5)  # 1/d
    V.tensor_tensor(
        m3(big1),
        s2[:, :, None].broadcast_to([P, M, M]),
        s2[:, None, :].broadcast_to([P, M, M]),
        OP.add,
    )
    S.activation(big1[:], big1[:], AF.Ln, scale=2.0)    # ln(2*(si^2+sj^2))
    S.activation(big1[:], big1[:], AF.Exp, scale=-0.5)  # gamma
    V.tensor_tensor(big3[:], big3[:], big1[:], OP.mult)  # arg = d*gamma
    S.activation(big1[:], big3[:], AF.Erf)               # erf(arg)
    V.tensor_tensor(big1[:], big1[:], big2[:], OP.mult)  # erf/d
    V.tensor_tensor(
        m3(big3),
        mB[:, :, None].broadcast_to([P, M, M]),
        msk[:, None, :].broadcast_to([P, M, M]),
        OP.mult,
    )                                                    # BOHR * mi*mj
    V.memset(_sap(big3[:, :], 0, [(M + 1, M)]), 0.0)     # zero its diagonal
    A3 = A[:, :].rearrange("p (i j) -> p i j", j=W)
    V.tensor_tensor(A3[:, :, 0:M], m3(big1), m3(big3), OP.mult)
    V.tensor_copy(_sap(A[:, :], 0, [(W + 1, M)]), diagc[:])
    V.tensor_copy(A3[:, :, M], msk[:])                   # u rhs
    V.scalar_tensor_tensor(A3[:, :, M + 1], chi[:], -1.0, msk[:], OP.mult, OP.mult)

    # ---- LDL^T factorization, rhs columns ride along ----
    for k in range(M):
        V.reciprocal(invd[:, k : k + 1], A3[:, k, k : k + 1])
        m = M - 1 - k
        if m > 0:
            wk = W - 1 - k
            T3 = big1[:, 0 : m * wk].rearrange("p (i j) -> p i j", j=wk)
            row_b = A3[:, k, k + 1 : W][:, None, :].broadcast_to([P, m, wk])
            col_b = A3[:, k + 1 : M, k][:, :, None].broadcast_to([P, m, wk])
            V.scalar_tensor_tensor(T3, row_b, invd[:, k : k + 1], col_b, OP.mult, OP.mult)
            V.tensor_tensor(
                A3[:, k + 1 : M, k + 1 : W], A3[:, k + 1 : M, k + 1 : W], T3, OP.subtract
            )
            V.tensor_scalar(
                A3[:, k, k + 1 : M], A3[:, k, k + 1 : M], invd[:, k : k + 1], None, OP.mult
            )

    # ---- diagonal scale (z -> w), backward substitution ----
    x3 = x[:, :].rearrange("p (r j) -> p r j", j=M)
    V.tensor_tensor(
        x3,
        A3[:, :, M : M + 2].transpose([0, 2, 1]),
        invd[:, None, :].broadcast_to([P, 2, M]),
        OP.mult,
    )
    for k in range(M - 1, 0, -1):
        V.tensor_scalar(negx[:], x3[:, :, k], -1.0, None, OP.mult)
        for r in range(2):
            V.scalar_tensor_tensor(
                x3[:, r, 0:k], A3[:, 0:k, k], negx[:, r : r + 1],
                x3[:, r, 0:k], OP.mult, OP.add,
            )

    # ---- lambda, charges ----
    V.tensor_reduce(sums2[:], x3, mybir.AxisListType.X, OP.add)
    V.tensor_tensor(lam[:], sums2[:, 1:2], qt[:], OP.subtract)
    V.reciprocal(su_r[:], sums2[:, 0:1])
    V.tensor_tensor(lam[:], lam[:], su_r[:], OP.mult)
    V.tensor_scalar(neglam[:], lam[:], -1.0, None, OP.mult)
    V.scalar_tensor_tensor(q[:], x3[:, 0, :], neglam[:], x3[:, 1, :], OP.mult, OP.add)

    # ---- energy ----
    V.tensor_tensor(junk[:], q[:], chi[:], OP.mult)
    V.tensor_reduce(e1[:], junk[:], mybir.AxisListType.X, OP.add)
    V.tensor_tensor(tq[:], q[:], hrd[:], OP.mult)
    V.tensor_tensor(junk[:], tq[:], q[:], OP.mult)
    V.tensor_reduce(e2[:], junk[:], mybir.AxisListType.X, OP.add)
    V.scalar_tensor_tensor(en[:], e2[:], BOHR, e1[:], OP.mult, OP.add)
    V.tensor_tensor(tq[:, 0:1], lam[:], qt[:], OP.mult)
    V.tensor_tensor(en[:], en[:], tq[:, 0:1], OP.add)
    V.tensor_scalar(en[:], en[:], -0.5, None, OP.mult)

    dma(d_charge[r0 : r0 + P, :], q[:])
    dma(d_energy[r0 : r0 + P, :], en[:])


def build_nc():
    import concourse.bass as bass
    from concourse import mybir, tile

    F = mybir.dt.float32
    nc = bass.Bass(trn_type="TRN2")
    d_pos = nc.dram_tensor("pos", [NB, 192], F, kind="ExternalInput")
    d_msk = nc.dram_tensor("msk", [NB, M], F, kind="ExternalInput")
    d_hrd = nc.dram_tensor("hrd", [NB, M], F, kind="ExternalInput")
    d_chi = nc.dram_tensor("chi", [NB, M], F, kind="ExternalInput")
    d_sig = nc.dram_tensor("sig", [NB, M], F, kind="ExternalInput")
    d_qt = nc.dram_tensor("qt", [NB, 1], F, kind="ExternalInput")
    d_charge = nc.dram_tensor("charge", [NB, M], F, kind="ExternalOutput")
    d_energy = nc.dram_tensor("energy", [NB, 1], F, kind="ExternalOutput")
    dram = (d_pos, d_msk, d_hrd, d_chi, d_sig, d_qt, d_charge, d_energy)

    with tile.TileContext(nc) as tc:
        with tc.tile_pool(name="pool", bufs=1) as pool:
            for blk in range(NBLK):
                _emit_block(nc, pool, blk, dram)

    _split_multiwait(nc, 1)
    return nc


def _get_prog():
    global _prog
    if _prog is None:
        _prog = build_nc()
    return _prog


def kernel(index, positions, hardness, electronegativity, sigma, total_charge):
    from concourse.bass_utils import run_bass_kernel_spmd

    index = np.asarray(index)
    B = index.shape[0]
    assert B == B_TOT and index.shape[1] == M
    msk = (index != -1).astype(np.float32)
    pos = np.ascontiguousarray(
        np.asarray(positions, dtype=np.float32).reshape(B, M * 3)
    )
    hrd = np.ascontiguousarray(np.asarray(hardness, dtype=np.float32))
    chi = np.ascontiguousarray(np.asarray(electronegativity, dtype=np.float32))
    sig = np.ascontiguousarray(np.asarray(sigma, dtype=np.float32))
    qt = np.ascontiguousarray(
        np.asarray(total_charge, dtype=np.float32).reshape(B, 1)
    )

    nc = _get_prog()
    in_maps = []
    for c in range(NCORES):
        s = slice(c * NB, (c + 1) * NB)
        in_maps.append(
            {
                "pos": np.ascontiguousarray(pos[s]),
                "msk": np.ascontiguousarray(msk[s]),
                "hrd": np.ascontiguousarray(hrd[s]),
                "chi": np.ascontiguousarray(chi[s]),
                "sig": np.ascontiguousarray(sig[s]),
                "qt": np.ascontiguousarray(qt[s]),
            }
        )
    res = run_bass_kernel_spmd(nc, in_maps, core_ids=list(range(NCORES)))
    charge = np.concatenate([res.results[c]["charge"] for c in range(NCORES)], 0)
    energy = np.concatenate([res.results[c]["energy"] for c in range(NCORES)], 0)[:, 0]
    return charge.astype(np.float32), energy.astype(np.float32)
